# revision 28
# baseline (speedup 1.0000x reference)
"""Trainium2 Bass kernel for nn_BackMapLayer (internal-coords -> cartesian backmap).

Reformulation (validated in sim_new.py against the reference):
  1) in-plane chain via cumsums (heading angles, bond vectors, positions)
  2) per-bond rotation quaternions g_p; Hamilton prefix products h_p
  3) final bonds = in-plane bonds rotated by h_p; positions = cumsum.

This version uses a work-efficient two-level scan with a parity-split (even/
odd scan position) data layout threaded through the whole pipeline:
  - trig activations write E|O split layouts directly (strided writes are
    cheap; all hot vector reads stay contiguous)
  - pair-reduce P_j = g_{2j} (x) g_{2j+1}, 7 Hillis-Steele rounds over 129
    entries (half the baseline width), then one expand h_{2j} = S_{j-1} (x)
    g_{2j}; h_{2j+1} = S_j comes straight from the scan
  - h components land in compact [E|O] component tiles via 3D split-write APs
    on the combines, so the bond-rotation phase reads contiguously
  - final position cumsums use pairsum+scan+recombine with strided writes
Data parallel over batch: 512 rows -> 8 cores x 64 rows; left chain lives on
partitions 64..127, right chain on 0..63 (row shard duplicated).
"""
import os
from contextlib import ExitStack

import numpy as np

import concourse.bacc as bacc
import concourse.bass as bass
import concourse.mybir as mybir
import concourse.tile as tile
from concourse import bass_utils
from concourse.dve_uop import (
    DISABLE, ENABLE, AluInp, AluOp, DelayInp, DveOpSpec, InpSel, OutPath,
    OutSel, Trigger, UopConfig,
)

# ---------------------------------------------------------------------------
# Hand-authored fused complex-multiply DVE ops (CMUL / CMULC) — unchanged from
# the baseline; registered into concourse.dve_ops at build time.
# ---------------------------------------------------------------------------


_REG = {}


class _FakeSpec:
    accum = None
    _leaves = frozenset()

    def __init__(self, reference):
        self.reference = reference


class HandDveOp:
    def __init__(self, name, uops, reference):
        self.name = name
        self.subdim = True
        self.spec = _FakeSpec(reference)
        self._uops = uops
        self._compiled = {}

    def compile(self, ver):
        if ver not in self._compiled:
            self._compiled[ver] = DveOpSpec(
                name=self.name, opcode=_REG[self.name], uops=self._uops,
                rd1_en=True)
        return self._compiled[ver]


def _base():
    u = UopConfig()
    u.enable_input(InpSel.SRC_0, 1)   # -> delay chain 0
    u.enable_input(InpSel.SRC_1, 2)   # -> delay chain 1
    u.require_inp0 = ENABLE
    u.require_inp1 = ENABLE
    for s in range(8):
        u.datapath_config[s].pass_through_delay(0, 1)
    u.enable_output(OutSel.ALU_OUT, OutPath.WR0_LO)
    u.trigger = (Trigger.SRC_TENSOR_DONE, Trigger.SUB_DIM_DONE, Trigger.NONE)
    return u


def _mk_E(conj: bool) -> UopConfig:
    u = _base()
    dp = u.datapath_config
    dp[0].enable_alu(AluOp.MULTIPLY, AluInp.PREV_DELAY_0, AluInp.PREV_DELAY_1)
    dp[1].enable_alu(AluOp.BYPASS, AluInp.PREV_ALU_OUT, AluInp.PREV_DELAY_0)
    dp[1].swap_enable = ENABLE
    dp[2].enable_alu(AluOp.BYPASS, AluInp.PREV_ALU_OUT, AluInp.PREV_DELAY_1)
    dp[2].swap_enable = ENABLE
    dp[3].enable_alu(AluOp.BYPASS, AluInp.CURR_ALU_OUT, AluInp.CURR_ALU_OUT)
    dp[4].enable_alu(AluOp.BYPASS, AluInp.CURR_ALU_OUT, AluInp.CURR_ALU_OUT)
    dp[4].enable_delay_from_src(DelayInp.PREV_ALU_OUT, 2)
    dp[5].enable_alu(AluOp.BYPASS, AluInp.PREV_ALU_OUT, AluInp.PREV_ALU_OUT)
    dp[5].pass_through_delay(2)
    dp[6].enable_alu(AluOp.BYPASS, AluInp.PREV_ALU_OUT, AluInp.PREV_ALU_OUT)
    dp[6].pass_through_delay(2)
    dp[7].enable_alu(AluOp.BYPASS, AluInp.PREV_DELAY_2, AluInp.PREV_DELAY_2)
    return u


def _mk_O(conj: bool) -> UopConfig:
    u = _base()
    dp = u.datapath_config
    dp[0].enable_alu(AluOp.MULTIPLY, AluInp.PREV_DELAY_0, AluInp.PREV_DELAY_1)
    dp[1].enable_alu(AluOp.MULTIPLY, AluInp.PREV_DELAY_1, AluInp.CURR_SWAP_OUT)
    dp[1].enable_delay_from_src(DelayInp.CURR_ALU_OUT, 2)
    dp[1].enable_delay_from_src(DelayInp.PREV_ALU_OUT, 3)
    dp[2].enable_alu(AluOp.MULTIPLY, AluInp.PREV_DELAY_0, AluInp.CURR_SWAP_OUT)
    dp[2].enable_delay_from_src(DelayInp.PREV_ALU_OUT, 4)
    dp[2].pass_through_delay(2, 3)
    dp[3].enable_alu(AluOp.ADD if conj else AluOp.SUBTRACT,
                     AluInp.PREV_DELAY_2, AluInp.PREV_DELAY_3)
    dp[3].enable_delay_from_src(DelayInp.PREV_ALU_OUT, 5)
    dp[3].pass_through_delay(4)
    if conj:
        dp[4].enable_alu(AluOp.SUBTRACT, AluInp.PREV_DELAY_5, AluInp.PREV_DELAY_4)
    else:
        dp[4].enable_alu(AluOp.ADD, AluInp.PREV_DELAY_4, AluInp.PREV_DELAY_5)
    dp[5].enable_alu(AluOp.BYPASS, AluInp.CURR_ALU_OUT, AluInp.CURR_ALU_OUT)
    dp[6].enable_alu(AluOp.BYPASS, AluInp.PREV_ALU_OUT, AluInp.PREV_ALU_OUT)
    dp[7].enable_alu(AluOp.BYPASS, AluInp.PREV_ALU_OUT, AluInp.PREV_ALU_OUT)
    return u


def _mk_warm():
    u = _base()
    u.require_inp0 = DISABLE
    u.require_inp1 = DISABLE
    u.out_enable = {p: DISABLE for p in u.out_enable}
    u.trigger = (Trigger.COUNT, Trigger.NONE, Trigger.NONE)
    u.repeat_count = 1
    u.next_uop = (1, 0, 0)
    return u


def _mk_uops(conj: bool):
    w = _mk_warm()
    e = _mk_E(conj); e.next_uop = (0, 2, 0)
    o = _mk_O(conj); o.next_uop = (0, 1, 0)
    return [w, e, o]


def _ref(conj):
    import numpy as np

    def reference(in0, in1, c0, c1, c2):
        x0 = np.asarray(in0, np.float32)
        x1 = np.asarray(in1, np.float32)
        flat0 = x0.reshape(x0.shape[0], -1)
        flat1 = x1.reshape(x1.shape[0], -1)
        n = flat0.shape[1] // 2
        ar, ai = flat0[:, 0::2], flat0[:, 1::2]
        br, bi = flat1[:, 0::2], flat1[:, 1::2]
        if conj:
            re = ar * br + ai * bi
            im = ai * br - ar * bi
        else:
            re = ar * br - ai * bi
            im = ar * bi + ai * br
        out = np.zeros_like(flat0)
        out[:, 2::2] = re[:, : n - 1]
        out[:, 3::2] = im[:, : n - 1]
        return out.reshape(x0.shape)

    return reference


def register():
    """Append CMUL/CMULC to the dve_ops registry (idempotent)."""
    import concourse.dve_ops as dve_ops

    ops = {}
    for name, conj in (("CMUL_ANT", False), ("CMULC_ANT", True)):
        if name in dve_ops._SUB_OPCODE_FOR_NAME:
            ops[name] = next(o for o in dve_ops.OPS if o.name == name)
            continue
        row = max(dve_ops._SUB_OPCODE_FOR_NAME.values()) + 1
        assert row < 0x20
        _REG[name] = row
        op = HandDveOp(name, _mk_uops(conj), _ref(conj))
        dve_ops._SUB_OPCODE_FOR_NAME[name] = row
        dve_ops.OPS.append(op)
        dve_ops.CUSTOM_DVE_SPECS[name] = op.spec
        ops[name] = op
    return ops["CMUL_ANT"], ops["CMULC_ANT"]


F32 = mybir.dt.float32
PI = float(np.pi)
TWO_PI = float(2.0 * np.pi)
MAGIC = float(1.5 * 2.0 ** 23)   # round-to-nearest magic for |x| < 2^22

NB = 64              # batch rows per core
N_CORES = 8
B_FULL = NB * N_CORES
N_ATOMS = 512
N_DIST = 511
N_ANG = 510
N_DIH = 509
RIGHT_SPLIT = 254
LEFT_SPLIT = 255
WQ = 260             # quaternion pair-tile width (130 CD pairs)

Alu = mybir.AluOpType
Act = mybir.ActivationFunctionType

R = slice(0, 64)     # right-chain partition rows
L = slice(64, 128)   # left-chain partition rows


def emit_backmap(ctx: ExitStack, tc: tile.TileContext, out_ap, dist_ap, ang_ap, dih_ap):
    nc = tc.nc
    pool = ctx.enter_context(tc.tile_pool(name="main", bufs=1))
    ppool = ctx.enter_context(tc.tile_pool(name="psum", bufs=1, space="PSUM"))
    CMUL, CMULC = register()

    def r3(ap):
        return ap.rearrange("p (s n) -> p s n", n=1)

    def split2(ap_2d, half):
        """[p, 2*half] contiguous AP -> [p, half, 2] split-layout write AP:
        elem k lands at col (k%2)*half + k//2."""
        return ap_2d.rearrange("p (c m) -> p m c", c=2)

    # ---------------- Phase 0: input DMAs first, then constants ------------
    angt = pool.tile([128, N_ANG], F32)
    diht = pool.tile([128, 510], F32)
    dt0 = pool.tile([64, N_DIST], F32, tag="dist0", name="dist0")
    nc.sync.dma_start(angt[0:32, :], ang_ap[0:32, :])
    nc.scalar.dma_start(angt[32:64, :], ang_ap[32:64, :])
    nc.gpsimd.dma_start(angt[64:96, :], ang_ap[0:32, :])
    nc.sync.dma_start(angt[96:128, :], ang_ap[32:64, :])
    nc.gpsimd.dma_start(diht[0:64, 0:N_DIH], dih_ap[:, :])
    nc.scalar.dma_start(diht[64:96, 0:N_DIH], dih_ap[0:32, :])
    nc.sync.dma_start(diht[96:128, 0:N_DIH], dih_ap[32:64, :])
    nc.sync.dma_start(dt0[:], dist_ap[0:64, :])

    bz = pool.tile([128, 1], F32)
    bpi2 = pool.tile([128, 1], F32)
    nc.vector.memset(bz[:], 0.0)
    nc.vector.memset(bpi2[:], PI / 2.0)
    warm = pool.tile([128, 1], F32)
    nc.scalar.activation(warm[:], bz[:, :], Act.Sin, bias=bz[:, :])

    ones = pool.tile([128, 512], F32)
    nc.vector.memset(ones[:], 1.0)
    ones_col = pool.tile([128, 1], F32)
    nc.vector.memset(ones_col[:], 1.0)
    ones_row = pool.tile([1, 128], F32)
    nc.vector.memset(ones_row[:], 1.0)
    nc.vector.memset(diht[:, 509:510], 0.0)

    # quaternion tiles + pads (gpsimd while idle at start)
    qaE = pool.tile([128, WQ], F32)
    qbE = pool.tile([128, WQ], F32)
    qaO = pool.tile([128, WQ], F32)
    qbO = pool.tile([128, WQ], F32)
    qaP = [pool.tile([128, WQ], F32, tag=f"qaP{i}", name=f"qaP{i}") for i in range(3)]
    qbP = [pool.tile([128, WQ], F32, tag=f"qbP{i}", name=f"qbP{i}") for i in range(3)]
    nc.gpsimd.memset(qbE[:], 0.0)
    nc.gpsimd.memset(qbO[:], 0.0)
    nc.gpsimd.memset(qaE[0:64, 254:260:2], 1.0)
    nc.gpsimd.memset(qaE[0:64, 255:260:2], 0.0)
    nc.gpsimd.memset(qaE[64:128, 256:260:2], 1.0)
    nc.gpsimd.memset(qaE[64:128, 257:260:2], 0.0)
    nc.gpsimd.memset(qaO[:, 254:260:2], 1.0)
    nc.gpsimd.memset(qaO[:, 255:260:2], 0.0)
    for i in range(3):
        nc.gpsimd.memset(qaP[i][:, 258:259], 1.0)
        nc.gpsimd.memset(qaP[i][:, 259:260], 0.0)
        nc.gpsimd.memset(qbP[i][:, 258:260], 0.0)
    nc.gpsimd.memset(qaP[0][:, 0:1], 1.0)
    nc.gpsimd.memset(qaP[0][:, 1:2], 0.0)
    nc.gpsimd.memset(qbP[0][:, 0:2], 0.0)

    # ---------------- Phase B: headings + range reduction ------------------
    integ = pool.tile([128, N_ANG], F32)
    nc.vector.tensor_scalar(integ[:], angt[:], -1.0, PI, Alu.mult, Alu.add)
    nc.vector.tensor_scalar_mul(integ[:, 1:N_ANG:2], integ[:, 1:N_ANG:2], -1.0)

    th = pool.tile([128, 512], F32)
    nc.vector.memset(th[:, 0:1], 0.0)
    nc.vector.memset(th[:, 511:512], 0.0)
    # NOTE: the reference's alternating sign on th (cols 2,4,...) and the
    # altsign flip on sinq's odd cols compose to a global -1 on sin in the
    # split layout — folded into the Sin activation scale below; cos is even.
    nc.vector.tensor_tensor_scan(th[:, 1:511], ones[:, 0:N_ANG], integ[:],
                                 0.0, Alu.mult, Alu.add)

    kf = pool.tile([128, 512], F32)
    nc.vector.tensor_scalar(kf[:], th[:], 1.0 / TWO_PI, MAGIC, Alu.mult, Alu.add)
    nc.vector.tensor_scalar_add(kf[:], kf[:], -MAGIC)
    thm = pool.tile([128, 512], F32)
    nc.vector.cody_waite_cascade(thm[:], th[:], kf[:],
                                 6.28125, 0.0019350051879882812,
                                 3.019916050561733e-07)
    cw = pool.tile([128, 512], F32)
    nc.vector.add_range_wrap(cw[:], thm[:], PI / 2.0, PI, TWO_PI)

    # ---------------- Phase A: lengths (split layout) ----------------------
    psl = ppool.tile([1, 512], F32)
    nc.tensor.matmul(psl[:, 0:N_DIST], ones_col[0:64, :], dt0[:],
                     start=True, stop=True)
    sb_lenS = pool.tile([1, 512], F32)
    nc.vector.memset(sb_lenS[0:1, 511:512], 0.0)
    psb = ppool.tile([128, 512], F32)

    # ---------------- trig activations with split-layout outputs -----------
    CtS = pool.tile([128, 510], F32)
    StS = pool.tile([128, 510], F32)
    nc.scalar.activation(split2(CtS[:, 0:510], 255), diht[:], Act.Sin,
                         bias=bz[:, :], scale=-0.5)
    nc.scalar.activation(split2(StS[:, 0:510], 255), diht[:], Act.Sin,
                         bias=bpi2[:, :], scale=0.5)

    # sfac / qa w-components on the vector engine: it would otherwise idle
    # here waiting for sinq, and this keeps the scalar queue free for the
    # sin/cos activations that gate everything downstream.
    sfacE = pool.tile([128, 128], F32)
    sfacO = pool.tile([128, 128], F32)
    nc.vector.tensor_copy(sfacE[R, 0:127], StS[R, 382:509])
    nc.vector.tensor_scalar_mul(sfacE[L, 0:128], StS[L, 127::-1], -1.0)
    nc.vector.tensor_copy(sfacO[R, 0:127], StS[R, 128:255])
    nc.vector.tensor_scalar_mul(sfacO[L, 0:127], StS[L, 381:254:-1], -1.0)
    nc.vector.tensor_copy(qaE[R, 0:254:2], CtS[R, 382:509])
    nc.vector.tensor_copy(qaE[L, 0:256:2], CtS[L, 127::-1])
    nc.scalar.copy(qaO[R, 0:254:2], CtS[R, 128:255])
    nc.scalar.copy(qaO[L, 0:254:2], CtS[L, 381:254:-1])

    sinqS = pool.tile([128, 512], F32)
    cosqS = pool.tile([128, 512], F32)
    nc.scalar.activation(split2(sinqS[:, 0:512], 256), thm[:], Act.Sin,
                         bias=bz[:, :], scale=-1.0)
    nc.scalar.activation(split2(cosqS[:, 0:512], 256), cw[:], Act.Sin,
                         bias=bz[:, :])
    nc.scalar.mul(sb_lenS[0:1, 0:256], psl[0:1, 0:511:2], 1.0 / 64.0)
    nc.scalar.mul(sb_lenS[0:1, 256:511], psl[0:1, 1:511:2], 1.0 / 64.0)
    nc.tensor.matmul(psb[:], ones_row[:], sb_lenS[0:1, :], start=True, stop=True)

    # ---------------- Phase C: g quaternion x/y components -----------------
    nc.vector.tensor_mul(qbE[R, 0:254:2], sfacE[R, 0:127], sinqS[R, 128:255])
    nc.vector.tensor_mul(qbE[L, 0:256:2], sfacE[L, 0:128], sinqS[L, 383:255:-1])
    nc.vector.tensor_mul(qbO[R, 0:254:2], sfacO[R, 0:127], sinqS[R, 384:511])
    nc.vector.tensor_mul(qbO[L, 0:254:2], sfacO[L, 0:127], sinqS[L, 127:0:-1])
    nc.vector.tensor_mul(qaE[R, 1:255:2], sfacE[R, 0:127], cosqS[R, 128:255])
    nc.vector.tensor_mul(qaE[L, 1:257:2], sfacE[L, 0:128], cosqS[L, 383:255:-1])
    nc.vector.tensor_mul(qaO[R, 1:255:2], sfacO[R, 0:127], cosqS[R, 384:511])
    nc.vector.tensor_mul(qaO[L, 1:255:2], sfacO[L, 0:127], cosqS[L, 127:0:-1])

    # ---------------- pair stage: P_j = g_{2j} (x) g_{2j+1} -----------------
    scrs = [[pool.tile([128, WQ], F32, tag=f"scr{j}_{i}", name=f"scr{j}_{i}")
             for i in range(4)] for j in range(2)]
    scr = scrs[1]
    nc.vector._custom_dve(CMUL, out=r3(scr[2][:, 0:258]),
                          in0=r3(qbO[:, 0:258]), in1=r3(qaE[:, 0:258]))
    nc.vector._custom_dve(CMULC, out=r3(scr[3][:, 0:258]),
                          in0=r3(qbE[:, 0:258]), in1=r3(qaO[:, 0:258]))
    nc.gpsimd.tensor_tensor(qbP[0][:, 2:258], scr[2][:, 2:258],
                            scr[3][:, 2:258], Alu.add)
    nc.vector._custom_dve(CMUL, out=r3(scr[0][:, 0:258]),
                          in0=r3(qaE[:, 0:258]), in1=r3(qaO[:, 0:258]))
    nc.vector._custom_dve(CMULC, out=r3(scr[1][:, 0:258]),
                          in0=r3(qbE[:, 0:258]), in1=r3(qbO[:, 0:258]))
    nc.vector.tensor_tensor(qaP[0][:, 2:258], scr[0][:, 2:258],
                            scr[1][:, 2:258], Alu.subtract)

    dxtS = pool.tile([128, 512], F32)
    dytS = pool.tile([128, 512], F32)
    cxE = pool.tile([128, 129], F32)
    cxO = pool.tile([128, 128], F32)
    cyE = pool.tile([128, 129], F32)
    cyO = pool.tile([128, 128], F32)
    bx = pool.tile([128, 256], F32)
    by = pool.tile([128, 256], F32)

    # ---------------- 7 Hillis-Steele rounds over 129 entries --------------
    cur_i = 0
    for si, s in enumerate([1, 2, 4, 8, 16, 32, 64]):
        scr = scrs[si % 2]
        nxt_i = 1 if cur_i != 1 else 2
        cA, cB = qaP[cur_i], qbP[cur_i]
        nA, nB = qaP[nxt_i], qbP[nxt_i]
        L2 = 260 - 2 * s
        lo = 2 * s
        oc = 2 * s - 2
        a1 = cA[:, 0:L2]
        b1 = cB[:, 0:L2]
        a2 = cA[:, lo:lo + L2]
        b2 = cB[:, lo:lo + L2]
        nc.scalar.copy(nA[:, 0:lo], cA[:, 0:lo])
        nc.scalar.copy(nB[:, 0:lo], cB[:, 0:lo])
        nc.vector._custom_dve(CMUL, out=r3(scr[2][:, oc:oc + L2]),
                              in0=r3(b2), in1=r3(a1))
        nc.vector._custom_dve(CMULC, out=r3(scr[3][:, oc:oc + L2]),
                              in0=r3(b1), in1=r3(a2))
        nc.gpsimd.tensor_tensor(nB[:, lo:258], scr[2][:, lo:258],
                                scr[3][:, lo:258], Alu.add)
        nc.vector._custom_dve(CMUL, out=r3(scr[0][:, oc:oc + L2]),
                              in0=r3(a1), in1=r3(a2))
        nc.vector._custom_dve(CMULC, out=r3(scr[1][:, oc:oc + L2]),
                              in0=r3(b1), in1=r3(b2))
        # fillers between scr0/scr1 and the A-combine hide the ~1.45us DVE
        # completion-ack (bond-vector work that must happen anyway)
        if si == 0:
            nc.vector.tensor_mul(dxtS[:], psb[:], cosqS[:])
        elif si == 1:
            nc.vector.tensor_mul(dytS[:], psb[:], sinqS[:])
        elif si == 2:
            nc.vector.tensor_tensor_scan(cxE[:, 0:129], ones[:, 0:129],
                                         dxtS[:, 0:129], 0.0, Alu.mult, Alu.add)
            nc.vector.tensor_copy(bx[R, 0:128], dxtS[R, 384:512])
        elif si == 3:
            nc.vector.tensor_tensor_scan(cxO[:, 0:128], ones[:, 0:128],
                                         dxtS[:, 256:384], 0.0, Alu.mult, Alu.add)
            nc.vector.tensor_copy(by[R, 0:128], dytS[R, 384:512])
        elif si == 4:
            nc.vector.tensor_tensor_scan(cyE[:, 0:129], ones[:, 0:129],
                                         dytS[:, 0:129], 0.0, Alu.mult, Alu.add)
            nc.vector.tensor_copy(bx[R, 128:255], dxtS[R, 129:256])
        elif si == 5:
            nc.vector.tensor_tensor_scan(cyO[:, 0:128], ones[:, 0:128],
                                         dytS[:, 256:384], 0.0, Alu.mult, Alu.add)
            nc.vector.tensor_copy(by[R, 128:255], dytS[R, 129:256])
        nc.vector.tensor_tensor(nA[:, lo:258], scr[0][:, lo:258],
                                scr[1][:, lo:258], Alu.subtract)
        cur_i = nxt_i

    SA, SB = qaP[cur_i], qbP[cur_i]

    # bonds to rotate, L-side (reversed + negated) on scalar; R-side copies
    # ran as round fillers on vector
    nc.scalar.mul(bx[L, 0:128], dxtS[L, 127::-1], -1.0)
    nc.scalar.mul(bx[L, 128:255], dxtS[L, 382:255:-1], -1.0)
    nc.scalar.mul(by[L, 0:128], dytS[L, 127::-1], -1.0)
    nc.scalar.mul(by[L, 128:255], dytS[L, 382:255:-1], -1.0)

    # anchors p1 = (R: xs[257], L: xs[255]) and mid atoms 255..257
    p1x = pool.tile([128, 1], F32)
    p1y = pool.tile([128, 1], F32)
    nc.scalar.activation(p1x[R, :], cxE[R, 128:129], Act.Identity,
                         bias=cxO[R, 127:128])
    nc.scalar.activation(p1x[L, :], cxE[L, 127:128], Act.Identity,
                         bias=cxO[L, 126:127])
    nc.scalar.activation(p1y[R, :], cyE[R, 128:129], Act.Identity,
                         bias=cyO[R, 127:128])
    nc.scalar.activation(p1y[L, :], cyE[L, 127:128], Act.Identity,
                         bias=cyO[L, 126:127])

    mid = pool.tile([128, 9], F32)
    nc.gpsimd.memset(mid[R, 2:9:3], 0.0)
    nc.scalar.activation(mid[R, 0:1], cxE[R, 127:128], Act.Identity,
                         bias=cxO[R, 126:127])
    nc.scalar.activation(mid[R, 3:4], cxE[R, 127:128], Act.Identity,
                         bias=cxO[R, 127:128])
    nc.scalar.copy(mid[R, 6:7], p1x[R, :])
    nc.scalar.activation(mid[R, 1:2], cyE[R, 127:128], Act.Identity,
                         bias=cyO[R, 126:127])
    nc.scalar.activation(mid[R, 4:5], cyE[R, 127:128], Act.Identity,
                         bias=cyO[R, 127:128])
    nc.scalar.copy(mid[R, 7:8], p1y[R, :])
    nc.sync.dma_start(out_ap[:, 255:258, :],
                      mid[R, 0:9].rearrange("p (a c) -> p a c", c=3))

    # ---------------- expand + h component extraction ----------------------
    hwx = pool.tile([128, 512], F32)
    hyz = pool.tile([128, 512], F32)
    # O half: h_{2i+1} = S_i straight from the final state (split copies on
    # gpsimd — idle here, and scalar's queue would gate Phase E later)
    nc.gpsimd.tensor_copy(split2(hwx[:, 0:512], 256)[:, 128:256, :], SA[:, 2:258])
    nc.gpsimd.tensor_copy(split2(hyz[:, 0:512], 256)[:, 128:256, :], SB[:, 2:258])
    # E half: h_{2j} = S_{j-1} (x) g_{2j} (pair 0 of S is the identity)
    scr = scrs[1]
    nc.vector._custom_dve(CMUL, out=r3(scr[2][:, 0:258]),
                          in0=r3(qbE[:, 0:258]), in1=r3(SA[:, 0:258]))
    nc.vector._custom_dve(CMULC, out=r3(scr[3][:, 0:258]),
                          in0=r3(SB[:, 0:258]), in1=r3(qaE[:, 0:258]))
    nc.vector._custom_dve(CMUL, out=r3(scr[0][:, 0:258]),
                          in0=r3(SA[:, 0:258]), in1=r3(qaE[:, 0:258]))
    nc.vector._custom_dve(CMULC, out=r3(scr[1][:, 0:258]),
                          in0=r3(SB[:, 0:258]), in1=r3(qbE[:, 0:258]))
    nc.vector.tensor_tensor(split2(hyz[:, 0:512], 256)[:, 0:128, :],
                            scr[2][:, 2:258], scr[3][:, 2:258], Alu.add)
    nc.vector.tensor_tensor(split2(hwx[:, 0:512], 256)[:, 0:128, :],
                            scr[0][:, 2:258], scr[1][:, 2:258], Alu.subtract)

    hw_ = hwx[:, 0:256]
    hx = hwx[:, 256:512]
    hy = hyz[:, 0:256]
    hz = hyz[:, 256:512]

    # ---------------- Phase E: rotate bonds by h ---------------------------
    # Vector owns the x/y chain, gpsimd owns the z chain (czt refactored as
    # hz*(hx*bx + hy*by) so it needs no vector-produced inputs). The only
    # crossings are tz (g->v, consumed late) and s3 (g->v rz, consumed last).
    # All temporaries are fresh tiles; emission keeps producer->consumer
    # distance >= 2 per engine so completion-acks stay hidden.
    _tln = [0]

    def tl():
        _tln[0] += 1
        return pool.tile([128, 256], F32, tag=f"rot{_tln[0]}",
                         name=f"rot{_tln[0]}")

    m3 = tl(); m4 = tl(); tz = tl(); g1 = tl(); g2 = tl(); g3 = tl()
    czt = tl(); m13 = tl(); s3 = tl()
    p1 = tl(); p2 = tl(); m5 = tl(); m6 = tl(); m7 = tl(); m8 = tl()
    m11 = tl(); m12 = tl(); cxt = tl(); cyt = tl()
    s1 = tl(); s2 = tl(); rx = tl(); ry = tl(); rz = tl()

    # vector chain (incl. tz so m5/m8 never wait on gpsimd); producer ->
    # consumer distance >= 2 everywhere on vector
    nc.vector.tensor_mul(p1[:], hz, by[:])
    nc.vector.tensor_mul(p2[:], hz, bx[:])
    nc.vector.tensor_mul(m3[:], hx, by[:])
    nc.vector.tensor_mul(m4[:], hy, bx[:])
    nc.vector.tensor_mul(m11[:], hw_, p1[:])
    nc.vector.tensor_mul(m12[:], hw_, p2[:])
    nc.vector.tensor_tensor(tz[:], m3[:], m4[:], Alu.subtract)
    nc.vector.tensor_mul(m6[:], hz, p2[:])
    nc.vector.tensor_mul(m7[:], hz, p1[:])
    nc.vector.tensor_mul(m5[:], hy, tz[:])
    nc.vector.tensor_mul(m8[:], hx, tz[:])
    nc.vector.tensor_mul(g1[:], hx, bx[:])
    nc.vector.tensor_mul(g2[:], hy, by[:])
    nc.vector.tensor_tensor(cxt[:], m5[:], m6[:], Alu.subtract)
    nc.vector.tensor_tensor(cyt[:], m7[:], m8[:], Alu.add)
    nc.vector.tensor_tensor(g3[:], g1[:], g2[:], Alu.add)
    nc.vector.tensor_mul(m13[:], hw_, tz[:])
    nc.vector.tensor_tensor(s1[:], cxt[:], m11[:], Alu.subtract)
    nc.vector.tensor_tensor(s2[:], m12[:], cyt[:], Alu.subtract)
    nc.vector.tensor_mul(czt[:], hz, g3[:])
    nc.vector.scalar_tensor_tensor(rx[:], s1[:], 2.0, bx[:], Alu.mult, Alu.add)
    nc.vector.scalar_tensor_tensor(ry[:], s2[:], 2.0, by[:], Alu.mult, Alu.add)
    nc.vector.tensor_tensor(s3[:], m13[:], czt[:], Alu.add)

    # ---------------- Phase F: pairsum + scan + recombine ------------------
    # vector compute pipelined with distance >= 2; odd copies on scalar
    asm_r = pool.tile([128, 762], F32)
    asm_l = pool.tile([128, 765], F32)
    sP = [pool.tile([128, 127], F32, tag=f"sP{i}", name=f"sP{i}") for i in range(3)]
    cOO = [pool.tile([128, 128], F32, tag=f"cOO{i}", name=f"cOO{i}") for i in range(3)]
    nc.scalar.copy(cOO[0][:, 0:1], p1x[:, :])
    nc.scalar.copy(cOO[1][:, 0:1], p1y[:, :])
    nc.gpsimd.memset(cOO[2][:, 0:1], 0.0)

    def adds(ci, rr):
        c = ci
        # R: atoms 258+2m (even, m<=126) / 259+2m (odd)
        nc.vector.tensor_tensor(asm_r[R, c:757 + c:6], cOO[ci][R, 0:127],
                                rr[R, 0:127], Alu.add)
        nc.vector.tensor_tensor(asm_l[L, 762 + c::-6], cOO[ci][L, 0:128],
                                rr[L, 0:128], Alu.add)
        nc.scalar.copy(asm_r[R, 3 + c:760 + c:6], cOO[ci][R, 1:128])
        nc.scalar.copy(asm_l[L, 759 + c::-6], cOO[ci][L, 1:128])

    # interleaved so every producer->consumer pair has >= 2 ops of distance
    nc.vector.tensor_tensor(sP[0][:], rx[:, 0:127], rx[:, 128:255], Alu.add)
    nc.vector.tensor_scalar_mul(rz[:], s3[:], 2.0)
    nc.vector.tensor_tensor(sP[1][:], ry[:, 0:127], ry[:, 128:255], Alu.add)
    nc.vector.tensor_tensor(sP[2][:], rz[:, 0:127], rz[:, 128:255], Alu.add)
    nc.vector.tensor_tensor_scan(cOO[0][:, 1:128], ones[:, 0:127],
                                 sP[0][:], p1x[:, :], Alu.mult, Alu.add)
    nc.vector.tensor_tensor_scan(cOO[1][:, 1:128], ones[:, 0:127],
                                 sP[1][:], p1y[:, :], Alu.mult, Alu.add)
    nc.vector.tensor_tensor_scan(cOO[2][:, 1:128], ones[:, 0:127],
                                 sP[2][:], 0.0, Alu.mult, Alu.add)
    adds(0, rx)
    adds(2, rz)
    adds(1, ry)

    nc.sync.dma_start(out_ap[0:32, 258:512, :],
                      asm_r[0:32, 0:762].rearrange("p (a c) -> p a c", c=3))
    nc.scalar.dma_start(out_ap[32:64, 258:512, :],
                        asm_r[32:64, 0:762].rearrange("p (a c) -> p a c", c=3))
    nc.gpsimd.dma_start(out_ap[0:32, 0:255, :],
                        asm_l[64:96, 0:765].rearrange("p (a c) -> p a c", c=3))
    nc.sync.dma_start(out_ap[32:64, 0:255, :],
                      asm_l[96:128, 0:765].rearrange("p (a c) -> p a c", c=3))


def build():
    nc = bacc.Bacc("TRN2", target_bir_lowering=False, debug=False)
    dist_t = nc.dram_tensor("dist", [B_FULL, N_DIST], F32, kind="ExternalInput")
    ang_t = nc.dram_tensor("ang", [NB, N_ANG], F32, kind="ExternalInput")
    dih_t = nc.dram_tensor("dih", [NB, N_DIH], F32, kind="ExternalInput")
    out_t = nc.dram_tensor("out", [NB, N_ATOMS, 3], F32, kind="ExternalOutput")
    with tile.TileContext(nc) as tc:
        with ExitStack() as ctx:
            emit_backmap(ctx, tc, out_t.ap(), dist_t.ap(), ang_t.ap(), dih_t.ap())
    nc.compile()
    return nc


_NC_CACHE = None


def kernel(distances, angles, dihedrals, left_split=255, right_split=254):
    global _NC_CACHE
    assert int(left_split) == LEFT_SPLIT and int(right_split) == RIGHT_SPLIT
    distances = np.ascontiguousarray(distances, dtype=np.float32)
    angles = np.ascontiguousarray(angles, dtype=np.float32)
    dihedrals = np.ascontiguousarray(dihedrals, dtype=np.float32)
    assert distances.shape == (B_FULL, N_DIST)
    if _NC_CACHE is None:
        _NC_CACHE = build()
    nc = _NC_CACHE
    in_maps = []
    for c in range(N_CORES):
        rows = slice(NB * c, NB * (c + 1))
        in_maps.append({
            "dist": distances,
            "ang": angles[rows],
            "dih": dihedrals[rows],
        })
    res = bass_utils.run_bass_kernel_spmd(
        nc, in_maps, core_ids=list(range(N_CORES)),
        trace=bool(int(os.environ.get("BK_TRACE", "0"))),
    )
    out = np.concatenate([r["out"] for r in res.results], axis=0)
    if res.exec_time_ns is not None:
        kernel.last_exec_time_ns = res.exec_time_ns
    return out


kernel.last_exec_time_ns = None


# revision 30
# speedup vs baseline: 1.0235x; 1.0235x over previous
"""Trainium2 Bass kernel for nn_BackMapLayer (internal-coords -> cartesian backmap).

Reformulation (validated in sim_new.py against the reference):
  1) in-plane chain via cumsums (heading angles, bond vectors, positions)
  2) per-bond rotation quaternions g_p; Hamilton prefix products h_p
  3) final bonds = in-plane bonds rotated by h_p; positions = cumsum.

This version uses a work-efficient two-level scan with a parity-split (even/
odd scan position) data layout threaded through the whole pipeline:
  - trig activations write E|O split layouts directly (strided writes are
    cheap; all hot vector reads stay contiguous)
  - pair-reduce P_j = g_{2j} (x) g_{2j+1}, 7 Hillis-Steele rounds over 129
    entries (half the baseline width), then one expand h_{2j} = S_{j-1} (x)
    g_{2j}; h_{2j+1} = S_j comes straight from the scan
  - h components land in compact [E|O] component tiles via 3D split-write APs
    on the combines, so the bond-rotation phase reads contiguously
  - final position cumsums use pairsum+scan+recombine with strided writes
Data parallel over batch: 512 rows -> 8 cores x 64 rows; left chain lives on
partitions 64..127, right chain on 0..63 (row shard duplicated).
"""
import os
from contextlib import ExitStack

import numpy as np

import concourse.bacc as bacc
import concourse.bass as bass
import concourse.mybir as mybir
import concourse.tile as tile
from concourse import bass_utils
from concourse.dve_uop import (
    DISABLE, ENABLE, AluInp, AluOp, DelayInp, DveOpSpec, InpSel, OutPath,
    OutSel, Trigger, UopConfig,
)

# ---------------------------------------------------------------------------
# Hand-authored fused complex-multiply DVE ops (CMUL / CMULC) — unchanged from
# the baseline; registered into concourse.dve_ops at build time.
# ---------------------------------------------------------------------------


_REG = {}


class _FakeSpec:
    accum = None
    _leaves = frozenset()

    def __init__(self, reference):
        self.reference = reference


class HandDveOp:
    def __init__(self, name, uops, reference):
        self.name = name
        self.subdim = True
        self.spec = _FakeSpec(reference)
        self._uops = uops
        self._compiled = {}

    def compile(self, ver):
        if ver not in self._compiled:
            self._compiled[ver] = DveOpSpec(
                name=self.name, opcode=_REG[self.name], uops=self._uops,
                rd1_en=True)
        return self._compiled[ver]


def _base():
    u = UopConfig()
    u.enable_input(InpSel.SRC_0, 1)   # -> delay chain 0
    u.enable_input(InpSel.SRC_1, 2)   # -> delay chain 1
    u.require_inp0 = ENABLE
    u.require_inp1 = ENABLE
    for s in range(8):
        u.datapath_config[s].pass_through_delay(0, 1)
    u.enable_output(OutSel.ALU_OUT, OutPath.WR0_LO)
    u.trigger = (Trigger.SRC_TENSOR_DONE, Trigger.SUB_DIM_DONE, Trigger.NONE)
    return u


def _mk_E(conj: bool) -> UopConfig:
    u = _base()
    dp = u.datapath_config
    dp[0].enable_alu(AluOp.MULTIPLY, AluInp.PREV_DELAY_0, AluInp.PREV_DELAY_1)
    dp[1].enable_alu(AluOp.BYPASS, AluInp.PREV_ALU_OUT, AluInp.PREV_DELAY_0)
    dp[1].swap_enable = ENABLE
    dp[2].enable_alu(AluOp.BYPASS, AluInp.PREV_ALU_OUT, AluInp.PREV_DELAY_1)
    dp[2].swap_enable = ENABLE
    dp[3].enable_alu(AluOp.BYPASS, AluInp.CURR_ALU_OUT, AluInp.CURR_ALU_OUT)
    dp[4].enable_alu(AluOp.BYPASS, AluInp.CURR_ALU_OUT, AluInp.CURR_ALU_OUT)
    dp[4].enable_delay_from_src(DelayInp.PREV_ALU_OUT, 2)
    dp[5].enable_alu(AluOp.BYPASS, AluInp.PREV_ALU_OUT, AluInp.PREV_ALU_OUT)
    dp[5].pass_through_delay(2)
    dp[6].enable_alu(AluOp.BYPASS, AluInp.PREV_ALU_OUT, AluInp.PREV_ALU_OUT)
    dp[6].pass_through_delay(2)
    dp[7].enable_alu(AluOp.BYPASS, AluInp.PREV_DELAY_2, AluInp.PREV_DELAY_2)
    return u


def _mk_O(conj: bool) -> UopConfig:
    u = _base()
    dp = u.datapath_config
    dp[0].enable_alu(AluOp.MULTIPLY, AluInp.PREV_DELAY_0, AluInp.PREV_DELAY_1)
    dp[1].enable_alu(AluOp.MULTIPLY, AluInp.PREV_DELAY_1, AluInp.CURR_SWAP_OUT)
    dp[1].enable_delay_from_src(DelayInp.CURR_ALU_OUT, 2)
    dp[1].enable_delay_from_src(DelayInp.PREV_ALU_OUT, 3)
    dp[2].enable_alu(AluOp.MULTIPLY, AluInp.PREV_DELAY_0, AluInp.CURR_SWAP_OUT)
    dp[2].enable_delay_from_src(DelayInp.PREV_ALU_OUT, 4)
    dp[2].pass_through_delay(2, 3)
    dp[3].enable_alu(AluOp.ADD if conj else AluOp.SUBTRACT,
                     AluInp.PREV_DELAY_2, AluInp.PREV_DELAY_3)
    dp[3].enable_delay_from_src(DelayInp.PREV_ALU_OUT, 5)
    dp[3].pass_through_delay(4)
    if conj:
        dp[4].enable_alu(AluOp.SUBTRACT, AluInp.PREV_DELAY_5, AluInp.PREV_DELAY_4)
    else:
        dp[4].enable_alu(AluOp.ADD, AluInp.PREV_DELAY_4, AluInp.PREV_DELAY_5)
    dp[5].enable_alu(AluOp.BYPASS, AluInp.CURR_ALU_OUT, AluInp.CURR_ALU_OUT)
    dp[6].enable_alu(AluOp.BYPASS, AluInp.PREV_ALU_OUT, AluInp.PREV_ALU_OUT)
    dp[7].enable_alu(AluOp.BYPASS, AluInp.PREV_ALU_OUT, AluInp.PREV_ALU_OUT)
    return u


def _mk_warm():
    u = _base()
    u.require_inp0 = DISABLE
    u.require_inp1 = DISABLE
    u.out_enable = {p: DISABLE for p in u.out_enable}
    u.trigger = (Trigger.COUNT, Trigger.NONE, Trigger.NONE)
    u.repeat_count = 1
    u.next_uop = (1, 0, 0)
    return u


def _mk_uops(conj: bool):
    w = _mk_warm()
    e = _mk_E(conj); e.next_uop = (0, 2, 0)
    o = _mk_O(conj); o.next_uop = (0, 1, 0)
    return [w, e, o]


def _ref(conj):
    import numpy as np

    def reference(in0, in1, c0, c1, c2):
        x0 = np.asarray(in0, np.float32)
        x1 = np.asarray(in1, np.float32)
        flat0 = x0.reshape(x0.shape[0], -1)
        flat1 = x1.reshape(x1.shape[0], -1)
        n = flat0.shape[1] // 2
        ar, ai = flat0[:, 0::2], flat0[:, 1::2]
        br, bi = flat1[:, 0::2], flat1[:, 1::2]
        if conj:
            re = ar * br + ai * bi
            im = ai * br - ar * bi
        else:
            re = ar * br - ai * bi
            im = ar * bi + ai * br
        out = np.zeros_like(flat0)
        out[:, 2::2] = re[:, : n - 1]
        out[:, 3::2] = im[:, : n - 1]
        return out.reshape(x0.shape)

    return reference


def register():
    """Append CMUL/CMULC to the dve_ops registry (idempotent)."""
    import concourse.dve_ops as dve_ops

    ops = {}
    for name, conj in (("CMUL_ANT", False), ("CMULC_ANT", True)):
        if name in dve_ops._SUB_OPCODE_FOR_NAME:
            ops[name] = next(o for o in dve_ops.OPS if o.name == name)
            continue
        row = max(dve_ops._SUB_OPCODE_FOR_NAME.values()) + 1
        assert row < 0x20
        _REG[name] = row
        op = HandDveOp(name, _mk_uops(conj), _ref(conj))
        dve_ops._SUB_OPCODE_FOR_NAME[name] = row
        dve_ops.OPS.append(op)
        dve_ops.CUSTOM_DVE_SPECS[name] = op.spec
        ops[name] = op
    return ops["CMUL_ANT"], ops["CMULC_ANT"]


F32 = mybir.dt.float32
PI = float(np.pi)
TWO_PI = float(2.0 * np.pi)
MAGIC = float(1.5 * 2.0 ** 23)   # round-to-nearest magic for |x| < 2^22

NB = 64              # batch rows per core
N_CORES = 8
B_FULL = NB * N_CORES
N_ATOMS = 512
N_DIST = 511
N_ANG = 510
N_DIH = 509
RIGHT_SPLIT = 254
LEFT_SPLIT = 255
WQ = 260             # quaternion pair-tile width (130 CD pairs)

Alu = mybir.AluOpType
Act = mybir.ActivationFunctionType

R = slice(0, 64)     # right-chain partition rows
L = slice(64, 128)   # left-chain partition rows


def emit_backmap(ctx: ExitStack, tc: tile.TileContext, out_ap, dist_ap, ang_ap, dih_ap):
    nc = tc.nc
    pool = ctx.enter_context(tc.tile_pool(name="main", bufs=1))
    ppool = ctx.enter_context(tc.tile_pool(name="psum", bufs=1, space="PSUM"))
    CMUL, CMULC = register()

    def r3(ap):
        return ap.rearrange("p (s n) -> p s n", n=1)

    def split2(ap_2d, half):
        """[p, 2*half] contiguous AP -> [p, half, 2] split-layout write AP:
        elem k lands at col (k%2)*half + k//2."""
        return ap_2d.rearrange("p (c m) -> p m c", c=2)

    # ---------------- Phase 0: input DMAs first, then constants ------------
    angt = pool.tile([128, N_ANG], F32)
    diht = pool.tile([128, 510], F32)
    dt0 = pool.tile([64, N_DIST], F32, tag="dist0", name="dist0")
    nc.sync.dma_start(angt[0:32, :], ang_ap[0:32, :])
    nc.scalar.dma_start(angt[32:64, :], ang_ap[32:64, :])
    nc.gpsimd.dma_start(angt[64:96, :], ang_ap[0:32, :])
    nc.sync.dma_start(angt[96:128, :], ang_ap[32:64, :])
    nc.gpsimd.dma_start(diht[0:64, 0:N_DIH], dih_ap[:, :])
    nc.scalar.dma_start(diht[64:96, 0:N_DIH], dih_ap[0:32, :])
    nc.sync.dma_start(diht[96:128, 0:N_DIH], dih_ap[32:64, :])
    nc.sync.dma_start(dt0[:], dist_ap[0:64, :])

    bz = pool.tile([128, 1], F32)
    bpi2 = pool.tile([128, 1], F32)
    nc.vector.memset(bz[:], 0.0)
    nc.vector.memset(bpi2[:], PI / 2.0)
    warm = pool.tile([128, 1], F32)
    nc.scalar.activation(warm[:], bz[:, :], Act.Sin, bias=bz[:, :])

    ones = pool.tile([128, 512], F32)
    nc.vector.memset(ones[:], 1.0)
    ones_col = pool.tile([128, 1], F32)
    nc.vector.memset(ones_col[:], 1.0)
    ones_row = pool.tile([1, 128], F32)
    nc.vector.memset(ones_row[:], 1.0)
    nc.vector.memset(diht[:, 509:510], 0.0)

    # quaternion tiles + pads (gpsimd while idle at start)
    qaE = pool.tile([128, WQ], F32)
    qbE = pool.tile([128, WQ], F32)
    qaO = pool.tile([128, WQ], F32)
    qbO = pool.tile([128, WQ], F32)
    qaP = [pool.tile([128, WQ], F32, tag=f"qaP{i}", name=f"qaP{i}") for i in range(3)]
    qbP = [pool.tile([128, WQ], F32, tag=f"qbP{i}", name=f"qbP{i}") for i in range(3)]
    nc.gpsimd.memset(qbE[:], 0.0)
    nc.gpsimd.memset(qbO[:], 0.0)
    nc.gpsimd.memset(qaE[0:64, 254:260:2], 1.0)
    nc.gpsimd.memset(qaE[0:64, 255:260:2], 0.0)
    nc.gpsimd.memset(qaE[64:128, 256:260:2], 1.0)
    nc.gpsimd.memset(qaE[64:128, 257:260:2], 0.0)
    nc.gpsimd.memset(qaO[:, 254:260:2], 1.0)
    nc.gpsimd.memset(qaO[:, 255:260:2], 0.0)
    for i in range(3):
        nc.gpsimd.memset(qaP[i][:, 258:259], 1.0)
        nc.gpsimd.memset(qaP[i][:, 259:260], 0.0)
        nc.gpsimd.memset(qbP[i][:, 258:260], 0.0)
    nc.gpsimd.memset(qaP[0][:, 0:1], 1.0)
    nc.gpsimd.memset(qaP[0][:, 1:2], 0.0)
    nc.gpsimd.memset(qbP[0][:, 0:2], 0.0)

    # ---------------- Phase B: headings + range reduction ------------------
    integ = pool.tile([128, N_ANG], F32)
    nc.vector.tensor_scalar(integ[:], angt[:], -1.0, PI, Alu.mult, Alu.add)
    nc.vector.tensor_scalar_mul(integ[:, 1:N_ANG:2], integ[:, 1:N_ANG:2], -1.0)

    th = pool.tile([128, 512], F32)
    nc.vector.memset(th[:, 0:1], 0.0)
    nc.vector.memset(th[:, 511:512], 0.0)
    # NOTE: the reference's alternating sign on th (cols 2,4,...) and the
    # altsign flip on sinq's odd cols compose to a global -1 on sin in the
    # split layout — folded into the Sin activation scale below; cos is even.
    nc.vector.tensor_tensor_scan(th[:, 1:511], ones[:, 0:N_ANG], integ[:],
                                 0.0, Alu.mult, Alu.add)

    kf = pool.tile([128, 512], F32)
    nc.vector.tensor_scalar(kf[:], th[:], 1.0 / TWO_PI, MAGIC, Alu.mult, Alu.add)
    nc.vector.tensor_scalar_add(kf[:], kf[:], -MAGIC)
    # single-step reduction: |k| <= 3 so the f32 2*pi error (~5e-7 rad) is
    # far below tolerance — no need for the 3-term Cody-Waite cascade
    thm = pool.tile([128, 512], F32)
    nc.vector.scalar_tensor_tensor(thm[:], kf[:], -TWO_PI, th[:],
                                   Alu.mult, Alu.add)
    cw = pool.tile([128, 512], F32)
    nc.vector.add_range_wrap(cw[:], thm[:], PI / 2.0, PI, TWO_PI)

    # ---------------- Phase A: lengths (split layout) ----------------------
    psl = ppool.tile([1, 512], F32)
    nc.tensor.matmul(psl[:, 0:N_DIST], ones_col[0:64, :], dt0[:],
                     start=True, stop=True)
    sb_lenS = pool.tile([1, 512], F32)
    nc.vector.memset(sb_lenS[0:1, 511:512], 0.0)
    psb = ppool.tile([128, 512], F32)

    # ---------------- trig activations with split-layout outputs -----------
    CtS = pool.tile([128, 510], F32)
    StS = pool.tile([128, 510], F32)
    nc.scalar.activation(split2(CtS[:, 0:510], 255), diht[:], Act.Sin,
                         bias=bz[:, :], scale=-0.5)
    nc.scalar.activation(split2(StS[:, 0:510], 255), diht[:], Act.Sin,
                         bias=bpi2[:, :], scale=0.5)

    # sfac / qa w-components on the vector engine: it would otherwise idle
    # here waiting for sinq, and this keeps the scalar queue free for the
    # sin/cos activations that gate everything downstream.
    sfacE = pool.tile([128, 128], F32)
    sfacO = pool.tile([128, 128], F32)
    nc.vector.tensor_copy(sfacE[R, 0:127], StS[R, 382:509])
    nc.vector.tensor_scalar_mul(sfacE[L, 0:128], StS[L, 127::-1], -1.0)
    nc.vector.tensor_copy(sfacO[R, 0:127], StS[R, 128:255])
    nc.vector.tensor_scalar_mul(sfacO[L, 0:127], StS[L, 381:254:-1], -1.0)
    nc.vector.tensor_copy(qaE[R, 0:254:2], CtS[R, 382:509])
    nc.vector.tensor_copy(qaE[L, 0:256:2], CtS[L, 127::-1])
    nc.scalar.copy(qaO[R, 0:254:2], CtS[R, 128:255])
    nc.scalar.copy(qaO[L, 0:254:2], CtS[L, 381:254:-1])

    sinqS = pool.tile([128, 512], F32)
    cosqS = pool.tile([128, 512], F32)
    nc.scalar.activation(split2(sinqS[:, 0:512], 256), thm[:], Act.Sin,
                         bias=bz[:, :], scale=-1.0)
    nc.scalar.activation(split2(cosqS[:, 0:512], 256), cw[:], Act.Sin,
                         bias=bz[:, :])
    nc.scalar.mul(sb_lenS[0:1, 0:256], psl[0:1, 0:511:2], 1.0 / 64.0)
    nc.scalar.mul(sb_lenS[0:1, 256:511], psl[0:1, 1:511:2], 1.0 / 64.0)
    nc.tensor.matmul(psb[:], ones_row[:], sb_lenS[0:1, :], start=True, stop=True)

    # ---------------- Phase C: g quaternion x/y components -----------------
    nc.vector.tensor_mul(qbE[R, 0:254:2], sfacE[R, 0:127], sinqS[R, 128:255])
    nc.vector.tensor_mul(qbE[L, 0:256:2], sfacE[L, 0:128], sinqS[L, 383:255:-1])
    nc.vector.tensor_mul(qbO[R, 0:254:2], sfacO[R, 0:127], sinqS[R, 384:511])
    nc.vector.tensor_mul(qbO[L, 0:254:2], sfacO[L, 0:127], sinqS[L, 127:0:-1])
    nc.vector.tensor_mul(qaE[R, 1:255:2], sfacE[R, 0:127], cosqS[R, 128:255])
    nc.vector.tensor_mul(qaE[L, 1:257:2], sfacE[L, 0:128], cosqS[L, 383:255:-1])
    nc.vector.tensor_mul(qaO[R, 1:255:2], sfacO[R, 0:127], cosqS[R, 384:511])
    nc.vector.tensor_mul(qaO[L, 1:255:2], sfacO[L, 0:127], cosqS[L, 127:0:-1])

    # ---------------- pair stage: P_j = g_{2j} (x) g_{2j+1} -----------------
    scrs = [[pool.tile([128, WQ], F32, tag=f"scr{j}_{i}", name=f"scr{j}_{i}")
             for i in range(4)] for j in range(2)]
    scr = scrs[1]
    nc.vector._custom_dve(CMUL, out=r3(scr[2][:, 0:258]),
                          in0=r3(qbO[:, 0:258]), in1=r3(qaE[:, 0:258]))
    nc.vector._custom_dve(CMULC, out=r3(scr[3][:, 0:258]),
                          in0=r3(qbE[:, 0:258]), in1=r3(qaO[:, 0:258]))
    nc.gpsimd.tensor_tensor(qbP[0][:, 2:258], scr[2][:, 2:258],
                            scr[3][:, 2:258], Alu.add)
    nc.vector._custom_dve(CMUL, out=r3(scr[0][:, 0:258]),
                          in0=r3(qaE[:, 0:258]), in1=r3(qaO[:, 0:258]))
    nc.vector._custom_dve(CMULC, out=r3(scr[1][:, 0:258]),
                          in0=r3(qbE[:, 0:258]), in1=r3(qbO[:, 0:258]))
    nc.vector.tensor_tensor(qaP[0][:, 2:258], scr[0][:, 2:258],
                            scr[1][:, 2:258], Alu.subtract)

    dxtS = pool.tile([128, 512], F32)
    dytS = pool.tile([128, 512], F32)
    cxE = pool.tile([128, 129], F32)
    cxO = pool.tile([128, 128], F32)
    cyE = pool.tile([128, 129], F32)
    cyO = pool.tile([128, 128], F32)
    bx = pool.tile([128, 256], F32)
    by = pool.tile([128, 256], F32)

    # ---------------- 7 Hillis-Steele rounds over 129 entries --------------
    cur_i = 0
    for si, s in enumerate([1, 2, 4, 8, 16, 32, 64]):
        scr = scrs[si % 2]
        nxt_i = 1 if cur_i != 1 else 2
        cA, cB = qaP[cur_i], qbP[cur_i]
        nA, nB = qaP[nxt_i], qbP[nxt_i]
        L2 = 260 - 2 * s
        lo = 2 * s
        oc = 2 * s - 2
        a1 = cA[:, 0:L2]
        b1 = cB[:, 0:L2]
        a2 = cA[:, lo:lo + L2]
        b2 = cB[:, lo:lo + L2]
        nc.scalar.copy(nA[:, 0:lo], cA[:, 0:lo])
        nc.scalar.copy(nB[:, 0:lo], cB[:, 0:lo])
        nc.vector._custom_dve(CMUL, out=r3(scr[2][:, oc:oc + L2]),
                              in0=r3(b2), in1=r3(a1))
        nc.vector._custom_dve(CMULC, out=r3(scr[3][:, oc:oc + L2]),
                              in0=r3(b1), in1=r3(a2))
        nc.gpsimd.tensor_tensor(nB[:, lo:258], scr[2][:, lo:258],
                                scr[3][:, lo:258], Alu.add)
        nc.vector._custom_dve(CMUL, out=r3(scr[0][:, oc:oc + L2]),
                              in0=r3(a1), in1=r3(a2))
        nc.vector._custom_dve(CMULC, out=r3(scr[1][:, oc:oc + L2]),
                              in0=r3(b1), in1=r3(b2))
        # fillers between scr0/scr1 and the A-combine hide the ~1.45us DVE
        # completion-ack (bond-vector work that must happen anyway)
        if si == 0:
            nc.vector.tensor_mul(dxtS[:], psb[:], cosqS[:])
        elif si == 1:
            nc.vector.tensor_mul(dytS[:], psb[:], sinqS[:])
        elif si == 2:
            nc.vector.tensor_tensor_scan(cxE[:, 0:129], ones[:, 0:129],
                                         dxtS[:, 0:129], 0.0, Alu.mult, Alu.add)
            nc.vector.tensor_copy(bx[R, 0:128], dxtS[R, 384:512])
        elif si == 3:
            nc.vector.tensor_tensor_scan(cxO[:, 0:128], ones[:, 0:128],
                                         dxtS[:, 256:384], 0.0, Alu.mult, Alu.add)
            nc.vector.tensor_copy(by[R, 0:128], dytS[R, 384:512])
        elif si == 4:
            nc.vector.tensor_tensor_scan(cyE[:, 0:129], ones[:, 0:129],
                                         dytS[:, 0:129], 0.0, Alu.mult, Alu.add)
            nc.vector.tensor_copy(bx[R, 128:255], dxtS[R, 129:256])
        elif si == 5:
            nc.vector.tensor_tensor_scan(cyO[:, 0:128], ones[:, 0:128],
                                         dytS[:, 256:384], 0.0, Alu.mult, Alu.add)
            nc.vector.tensor_copy(by[R, 128:255], dytS[R, 129:256])
        nc.vector.tensor_tensor(nA[:, lo:258], scr[0][:, lo:258],
                                scr[1][:, lo:258], Alu.subtract)
        cur_i = nxt_i

    SA, SB = qaP[cur_i], qbP[cur_i]

    # bonds to rotate, L-side (reversed + negated) on scalar; R-side copies
    # ran as round fillers on vector
    nc.scalar.mul(bx[L, 0:128], dxtS[L, 127::-1], -1.0)
    nc.scalar.mul(bx[L, 128:255], dxtS[L, 382:255:-1], -1.0)
    nc.scalar.mul(by[L, 0:128], dytS[L, 127::-1], -1.0)
    nc.scalar.mul(by[L, 128:255], dytS[L, 382:255:-1], -1.0)

    # anchors p1 = (R: xs[257], L: xs[255]) and mid atoms 255..257
    p1x = pool.tile([128, 1], F32)
    p1y = pool.tile([128, 1], F32)
    nc.scalar.activation(p1x[R, :], cxE[R, 128:129], Act.Identity,
                         bias=cxO[R, 127:128])
    nc.scalar.activation(p1x[L, :], cxE[L, 127:128], Act.Identity,
                         bias=cxO[L, 126:127])
    nc.scalar.activation(p1y[R, :], cyE[R, 128:129], Act.Identity,
                         bias=cyO[R, 127:128])
    nc.scalar.activation(p1y[L, :], cyE[L, 127:128], Act.Identity,
                         bias=cyO[L, 126:127])

    mid = pool.tile([128, 9], F32)
    nc.gpsimd.memset(mid[R, 2:9:3], 0.0)
    nc.scalar.activation(mid[R, 0:1], cxE[R, 127:128], Act.Identity,
                         bias=cxO[R, 126:127])
    nc.scalar.activation(mid[R, 3:4], cxE[R, 127:128], Act.Identity,
                         bias=cxO[R, 127:128])
    nc.scalar.copy(mid[R, 6:7], p1x[R, :])
    nc.scalar.activation(mid[R, 1:2], cyE[R, 127:128], Act.Identity,
                         bias=cyO[R, 126:127])
    nc.scalar.activation(mid[R, 4:5], cyE[R, 127:128], Act.Identity,
                         bias=cyO[R, 127:128])
    nc.scalar.copy(mid[R, 7:8], p1y[R, :])
    nc.sync.dma_start(out_ap[:, 255:258, :],
                      mid[R, 0:9].rearrange("p (a c) -> p a c", c=3))

    # ---------------- expand + h component extraction ----------------------
    hwx = pool.tile([128, 512], F32)
    hyz = pool.tile([128, 512], F32)
    # O half: h_{2i+1} = S_i straight from the final state (split copies)
    nc.scalar.copy(split2(hwx[:, 0:512], 256)[:, 128:256, :], SA[:, 2:258])
    nc.scalar.copy(split2(hyz[:, 0:512], 256)[:, 128:256, :], SB[:, 2:258])
    # E half: h_{2j} = S_{j-1} (x) g_{2j} (pair 0 of S is the identity)
    scr = scrs[1]
    nc.vector._custom_dve(CMUL, out=r3(scr[2][:, 0:258]),
                          in0=r3(qbE[:, 0:258]), in1=r3(SA[:, 0:258]))
    nc.vector._custom_dve(CMULC, out=r3(scr[3][:, 0:258]),
                          in0=r3(SB[:, 0:258]), in1=r3(qaE[:, 0:258]))
    nc.vector._custom_dve(CMUL, out=r3(scr[0][:, 0:258]),
                          in0=r3(SA[:, 0:258]), in1=r3(qaE[:, 0:258]))
    nc.vector._custom_dve(CMULC, out=r3(scr[1][:, 0:258]),
                          in0=r3(SB[:, 0:258]), in1=r3(qbE[:, 0:258]))
    nc.vector.tensor_tensor(split2(hyz[:, 0:512], 256)[:, 0:128, :],
                            scr[2][:, 2:258], scr[3][:, 2:258], Alu.add)
    nc.vector.tensor_tensor(split2(hwx[:, 0:512], 256)[:, 0:128, :],
                            scr[0][:, 2:258], scr[1][:, 2:258], Alu.subtract)

    hw_ = hwx[:, 0:256]
    hx = hwx[:, 256:512]
    hy = hyz[:, 0:256]
    hz = hyz[:, 256:512]

    # ---------------- Phase E: rotate bonds by h ---------------------------
    # Vector owns the x/y chain, gpsimd owns the z chain (czt refactored as
    # hz*(hx*bx + hy*by) so it needs no vector-produced inputs). The only
    # crossings are tz (g->v, consumed late) and s3 (g->v rz, consumed last).
    # All temporaries are fresh tiles; emission keeps producer->consumer
    # distance >= 2 per engine so completion-acks stay hidden.
    _tln = [0]

    def tl():
        _tln[0] += 1
        return pool.tile([128, 256], F32, tag=f"rot{_tln[0]}",
                         name=f"rot{_tln[0]}")

    m3 = tl(); m4 = tl(); tz = tl(); g1 = tl(); g2 = tl(); g3 = tl()
    czt = tl(); m13 = tl(); s3 = tl()
    p1 = tl(); p2 = tl(); m5 = tl(); m6 = tl(); m7 = tl(); m8 = tl()
    m11 = tl(); m12 = tl(); cxt = tl(); cyt = tl()
    s1 = tl(); s2 = tl(); rx = tl(); ry = tl(); rz = tl()

    # vector chain (incl. tz so m5/m8 never wait on gpsimd); producer ->
    # consumer distance >= 2 everywhere on vector
    nc.vector.tensor_mul(p1[:], hz, by[:])
    nc.vector.tensor_mul(p2[:], hz, bx[:])
    nc.vector.tensor_mul(m3[:], hx, by[:])
    nc.vector.tensor_mul(m4[:], hy, bx[:])
    nc.vector.tensor_mul(m11[:], hw_, p1[:])
    nc.vector.tensor_mul(m12[:], hw_, p2[:])
    nc.vector.tensor_tensor(tz[:], m3[:], m4[:], Alu.subtract)
    nc.vector.tensor_mul(m6[:], hz, p2[:])
    nc.vector.tensor_mul(m7[:], hz, p1[:])
    nc.vector.tensor_mul(m5[:], hy, tz[:])
    nc.vector.tensor_mul(m8[:], hx, tz[:])
    nc.vector.tensor_mul(g1[:], hx, bx[:])
    nc.vector.tensor_mul(g2[:], hy, by[:])
    nc.vector.tensor_tensor(cxt[:], m5[:], m6[:], Alu.subtract)
    nc.vector.tensor_tensor(cyt[:], m7[:], m8[:], Alu.add)
    nc.vector.tensor_tensor(g3[:], g1[:], g2[:], Alu.add)
    nc.vector.tensor_mul(m13[:], hw_, tz[:])
    nc.vector.tensor_tensor(s1[:], cxt[:], m11[:], Alu.subtract)
    nc.vector.tensor_tensor(s2[:], m12[:], cyt[:], Alu.subtract)
    nc.vector.tensor_mul(czt[:], hz, g3[:])
    nc.vector.scalar_tensor_tensor(rx[:], s1[:], 2.0, bx[:], Alu.mult, Alu.add)
    nc.vector.scalar_tensor_tensor(ry[:], s2[:], 2.0, by[:], Alu.mult, Alu.add)
    nc.vector.tensor_tensor(s3[:], m13[:], czt[:], Alu.add)

    # ---------------- Phase F: pairsum + scan + recombine ------------------
    # vector compute pipelined with distance >= 2; odd copies on scalar
    asm_r = pool.tile([128, 762], F32)
    asm_l = pool.tile([128, 765], F32)
    sP = [pool.tile([128, 127], F32, tag=f"sP{i}", name=f"sP{i}") for i in range(3)]
    cOO = [pool.tile([128, 128], F32, tag=f"cOO{i}", name=f"cOO{i}") for i in range(3)]
    nc.scalar.copy(cOO[0][:, 0:1], p1x[:, :])
    nc.scalar.copy(cOO[1][:, 0:1], p1y[:, :])
    nc.gpsimd.memset(cOO[2][:, 0:1], 0.0)

    def adds(ci, rr):
        c = ci
        # R: atoms 258+2m (even, m<=126) / 259+2m (odd)
        nc.vector.tensor_tensor(asm_r[R, c:757 + c:6], cOO[ci][R, 0:127],
                                rr[R, 0:127], Alu.add)
        nc.vector.tensor_tensor(asm_l[L, 762 + c::-6], cOO[ci][L, 0:128],
                                rr[L, 0:128], Alu.add)
        nc.scalar.copy(asm_r[R, 3 + c:760 + c:6], cOO[ci][R, 1:128])
        nc.scalar.copy(asm_l[L, 759 + c::-6], cOO[ci][L, 1:128])

    # interleaved so every producer->consumer pair has >= 2 ops of distance
    nc.vector.tensor_tensor(sP[0][:], rx[:, 0:127], rx[:, 128:255], Alu.add)
    nc.vector.tensor_scalar_mul(rz[:], s3[:], 2.0)
    nc.vector.tensor_tensor(sP[1][:], ry[:, 0:127], ry[:, 128:255], Alu.add)
    nc.vector.tensor_tensor(sP[2][:], rz[:, 0:127], rz[:, 128:255], Alu.add)
    nc.vector.tensor_tensor_scan(cOO[0][:, 1:128], ones[:, 0:127],
                                 sP[0][:], p1x[:, :], Alu.mult, Alu.add)
    nc.vector.tensor_tensor_scan(cOO[1][:, 1:128], ones[:, 0:127],
                                 sP[1][:], p1y[:, :], Alu.mult, Alu.add)
    nc.vector.tensor_tensor_scan(cOO[2][:, 1:128], ones[:, 0:127],
                                 sP[2][:], 0.0, Alu.mult, Alu.add)
    adds(0, rx)
    adds(2, rz)
    adds(1, ry)

    nc.sync.dma_start(out_ap[0:32, 258:512, :],
                      asm_r[0:32, 0:762].rearrange("p (a c) -> p a c", c=3))
    nc.scalar.dma_start(out_ap[32:64, 258:512, :],
                        asm_r[32:64, 0:762].rearrange("p (a c) -> p a c", c=3))
    nc.gpsimd.dma_start(out_ap[0:32, 0:255, :],
                        asm_l[64:96, 0:765].rearrange("p (a c) -> p a c", c=3))
    nc.sync.dma_start(out_ap[32:64, 0:255, :],
                      asm_l[96:128, 0:765].rearrange("p (a c) -> p a c", c=3))


def build():
    nc = bacc.Bacc("TRN2", target_bir_lowering=False, debug=False)
    dist_t = nc.dram_tensor("dist", [B_FULL, N_DIST], F32, kind="ExternalInput")
    ang_t = nc.dram_tensor("ang", [NB, N_ANG], F32, kind="ExternalInput")
    dih_t = nc.dram_tensor("dih", [NB, N_DIH], F32, kind="ExternalInput")
    out_t = nc.dram_tensor("out", [NB, N_ATOMS, 3], F32, kind="ExternalOutput")
    with tile.TileContext(nc) as tc:
        with ExitStack() as ctx:
            emit_backmap(ctx, tc, out_t.ap(), dist_t.ap(), ang_t.ap(), dih_t.ap())
    nc.compile()
    return nc


_NC_CACHE = None


def kernel(distances, angles, dihedrals, left_split=255, right_split=254):
    global _NC_CACHE
    assert int(left_split) == LEFT_SPLIT and int(right_split) == RIGHT_SPLIT
    distances = np.ascontiguousarray(distances, dtype=np.float32)
    angles = np.ascontiguousarray(angles, dtype=np.float32)
    dihedrals = np.ascontiguousarray(dihedrals, dtype=np.float32)
    assert distances.shape == (B_FULL, N_DIST)
    if _NC_CACHE is None:
        _NC_CACHE = build()
    nc = _NC_CACHE
    in_maps = []
    for c in range(N_CORES):
        rows = slice(NB * c, NB * (c + 1))
        in_maps.append({
            "dist": distances,
            "ang": angles[rows],
            "dih": dihedrals[rows],
        })
    res = bass_utils.run_bass_kernel_spmd(
        nc, in_maps, core_ids=list(range(N_CORES)),
        trace=bool(int(os.environ.get("BK_TRACE", "0"))),
    )
    out = np.concatenate([r["out"] for r in res.results], axis=0)
    if res.exec_time_ns is not None:
        kernel.last_exec_time_ns = res.exec_time_ns
    return out


kernel.last_exec_time_ns = None
